# revision 2
# baseline (speedup 1.0000x reference)
"""PSLoRA linear layer on 8 Trainium2 NeuronCores (Bass/Tile, bf16).

out[b] = x[b] @ W.T + bias + 0.5 * (x[b] @ lora_A[idx[b]]) @ lora_B.T

Sharding: data-parallel over batch (B=8 -> one batch element per core).
The LoRA update is rank-32 with only 5 distinct labelers, so it is folded
into the weights on the host: M_i = W.T + 0.5 * lora_A[i] @ lora_B.T
(one 4096x32x4096 GEMM per unique labeler). Each core then runs a plain
GEMM out = x[b] @ M_{idx[b]} with the bias added during PSUM eviction on
the vector engine, so the tensor engine does exactly the 4096 N=512
base matmuls and nothing else.

Device loop per core: 2 s-halves (x half resident in SBUF, bf16, 8 MiB);
per half: 8 output panels of 512 columns, each accumulating 32 K-tiles
across 8 PSUM banks (one per 128-row s-block), evicted via DVE
tensor_add (+bias) to SBUF and DMA'd out. Weight tiles are pre-tiled
contiguously on host ([OB, KT, 128, 512] bf16) for clean descriptors.
"""
import sys
sys.path.insert(0, "/opt/trn_rl_repo")
import numpy as np

B, S, DIN, DOUT, R = 8, 2048, 4096, 4096, 32
LORA_SCALING = 16 / 32
KT = DIN // 128          # 32 contraction tiles
HALF = 1024              # s rows per resident half
NH = S // HALF
SBH = HALF // 128        # s-blocks per half
OB = DOUT // 512         # output panels
N_CORES = 8

_cache = {}


def _build(hw_loop=1):
    import concourse.bacc as bacc
    import concourse.mybir as mybir
    from concourse.tile import TileContext

    BF16 = mybir.dt.bfloat16
    F32 = mybir.dt.float32

    nc = bacc.Bacc()
    xT = nc.dram_tensor("xT", [DIN, S], BF16, kind="ExternalInput")
    WT = nc.dram_tensor("WT", [OB, KT, 128, 512], BF16, kind="ExternalInput")
    BR = nc.dram_tensor("BR", [128, DOUT], F32, kind="ExternalInput")
    out = nc.dram_tensor("out", [S, DOUT], F32, kind="ExternalOutput")

    with TileContext(nc) as tc:
        with (
            tc.tile_pool(name="xp", bufs=KT) as xp,
            tc.tile_pool(name="wp", bufs=12) as wp,
            tc.tile_pool(name="cp", bufs=1) as cp,
            tc.tile_pool(name="op", bufs=8) as op_,
            tc.tile_pool(name="pp", bufs=1, space="PSUM") as pp,
        ):
            br = cp.tile([128, DOUT], F32, name="br")
            nc.sync.dma_start(br, BR[:, :])

            def body():
                for h in range(NH):
                    xt = []
                    for k in range(KT):
                        t = xp.tile([128, HALF], BF16, name="xq")
                        nc.sync.dma_start(
                            t, xT[k * 128:(k + 1) * 128,
                                  h * HALF:(h + 1) * HALF])
                        xt.append(t)
                    for ob in range(OB):
                        ps = [pp.tile([128, 512], F32, name=f"ps{sb}")
                              for sb in range(SBH)]
                        for k in range(KT):
                            wt = wp.tile([128, 512], BF16, name="wt")
                            nc.sync.dma_start(wt, WT[ob, k, :, :])
                            for sb in range(SBH):
                                nc.tensor.matmul(
                                    ps[sb],
                                    lhsT=xt[k][:, sb * 128:(sb + 1) * 128],
                                    rhs=wt, start=(k == 0), stop=(k == KT - 1))
                        for sb in range(SBH):
                            ot = op_.tile([128, 512], F32, name="ot")
                            nc.vector.tensor_add(
                                ot, ps[sb], br[:, ob * 512:(ob + 1) * 512])
                            nc.scalar.dma_start(
                                out[h * HALF + sb * 128:
                                    h * HALF + (sb + 1) * 128,
                                    ob * 512:(ob + 1) * 512], ot)

            if hw_loop > 1:
                with tc.For_i(0, hw_loop, 1):
                    body()
            else:
                body()
    nc.finalize()
    return nc


def _prep_in_maps(input, weight, bias, lora_A, lora_B, labeler_index):
    import ml_dtypes
    bf16 = np.dtype(ml_dtypes.bfloat16)

    x = np.asarray(input, dtype=np.float32)
    W = np.asarray(weight, dtype=np.float32)
    bias = np.asarray(bias, dtype=np.float32)
    lA = np.asarray(lora_A, dtype=np.float32)
    lB = np.asarray(lora_B, dtype=np.float32)
    idx = np.asarray(labeler_index).astype(np.int64)

    WTf = np.ascontiguousarray(W.T)                    # [DIN, DOUT]
    lBTs = (LORA_SCALING * lB.T).astype(np.float32)    # [R, DOUT]
    # per-unique-labeler folded + tiled weights [OB, KT, 128, 512] bf16
    wtiles = {}
    for i in np.unique(idx):
        M = WTf + lA[i] @ lBTs
        wtiles[i] = np.ascontiguousarray(
            M.reshape(KT, 128, OB, 512).transpose(2, 0, 1, 3)).astype(bf16)
    br = np.ascontiguousarray(np.broadcast_to(bias, (128, DOUT)),
                              dtype=np.float32)

    in_maps = []
    for b in range(B):
        xTb = np.ascontiguousarray(x[b].T).astype(bf16)
        in_maps.append({"xT": xTb, "WT": wtiles[int(idx[b])], "BR": br})
    return in_maps


def kernel(input, weight, bias, lora_A, lora_B, labeler_index):
    from concourse import bass_utils

    in_maps = _prep_in_maps(input, weight, bias, lora_A, lora_B, labeler_index)
    if "nc" not in _cache:
        _cache["nc"] = _build()
    last_err = None
    for attempt in range(3):
        try:
            res = bass_utils.run_bass_kernel_spmd(
                _cache["nc"], in_maps, core_ids=list(range(N_CORES)))
            return np.stack([res.results[b]["out"] for b in range(B)])
        except Exception as e:  # transient NRT wedge from a prior crashed run
            last_err = e
            if "UNRECOVERABLE" not in str(e) and "UNAVAILABLE" not in str(e):
                raise
    raise last_err
